# revision 24
# baseline (speedup 1.0000x reference)
"""Trainium2 Bass kernel for PhysicalHarmonicResonanceSystem.

Math: spectrum[b,d] = freq[d] * (1 + 0.1*sin(ps[idx[d]] + 2*pi*tm_b)),
row-normalized to energy_preservation, idx = arange(D) % 50. Using the
angle-addition identity, each row is a rank-3 combination of three [D]
basis vectors:

    row_b = A + cos(th_b)*B + sin(th_b)*C
    A = freq, B = 0.1*freq*sin(ps[idx]), C = 0.1*freq*cos(ps[idx])

so ||row_b||^2 collapses to 6 Gram scalars of (A,B,C) (computed in
50-element space with occurrence counts), and the normalized output is a
[B,3] @ [3,D] matmul. The [3,D] basis is never materialized: since
idx repeats every 50 columns, the matmul's moving operand reads the
[3,50] table through a stride-0 access pattern in 500-column chunks.
The per-(row-tile, chunk) K=3 matmuls are packed 4-per-PE-array via
tile_position row groups. The kernel is bound by writing the 128MB
output to HBM (~47us/core at ~360GB/s).

Sharding: data-parallel over the batch dim, 512 rows per core x 8 cores.
"""

import numpy as np

N_PRIMES = 50
BATCH = 4096
DIM = 8192
N_CORES = 8
B_LOC = BATCH // N_CORES  # 512
P = 128
NT = B_LOC // P  # 4 row tiles per core
GOLDEN = (1.0 + np.sqrt(5.0)) / 2.0
BASE_SCALE = float(GOLDEN * np.pi / DIM)
MAGIC = float(1.5 * 2**23)  # fp32 round-to-nearest via add/sub

# matmul chunks: (col_offset, n_cols, n_pattern_reps or None for tail)
_CHUNKS = []
_off = 0
while _off + 500 <= DIM:
    _CHUNKS.append((_off, 500, 10))
    _off += 500
# tail: 8000..8192 = 192 cols = 3*50 + 42, pattern-aligned at 8000%50==0
_CHUNKS.append((_off, 150, 3))
_CHUNKS.append((_off + 150, 42, None))

_CACHE = {}


def _build():
    from contextlib import ExitStack

    import concourse.bacc as bacc
    import concourse.bass as bass
    import concourse.tile as tile
    from concourse import mybir
    from concourse.masks import make_identity

    f32 = mybir.dt.float32
    i32 = mybir.dt.int32
    AF = mybir.ActivationFunctionType
    Op = mybir.AluOpType

    nc = bacc.Bacc()
    th = nc.dram_tensor("token_hash", [B_LOC], i32, kind="ExternalInput")
    primes = nc.dram_tensor("primes", [N_PRIMES], i32, kind="ExternalInput")
    rs = nc.dram_tensor("resonance_scales", [N_PRIMES], f32, kind="ExternalInput")
    psh = nc.dram_tensor("phase_shifts", [N_PRIMES], f32, kind="ExternalInput")
    ep = nc.dram_tensor("energy_preservation", [1], f32, kind="ExternalInput")
    out = nc.dram_tensor("out", [B_LOC, DIM], f32, kind="ExternalOutput")

    # occurrence count of each prime slot in idx = arange(DIM) % 50
    cnt = np.full(N_PRIMES, DIM // N_PRIMES, dtype=np.float32)
    cnt[: DIM % N_PRIMES] += 1.0
    cnt_d = nc.inline_tensor(cnt, name="cntw")

    with ExitStack() as ctx:
        tc = ctx.enter_context(tile.TileContext(nc))
        consts = ctx.enter_context(tc.tile_pool(name="consts", bufs=1))
        rowp = ctx.enter_context(tc.tile_pool(name="rowp", bufs=1))
        outp = ctx.enter_context(tc.tile_pool(name="outp", bufs=2))
        psum = ctx.enter_context(tc.tile_pool(name="psum", bufs=6, space="PSUM"))
        psum_t = ctx.enter_context(tc.tile_pool(name="psum_t", bufs=1, space="PSUM"))

        ident = consts.tile([P, P], f32)
        make_identity(nc, ident[:])
        halfpi = consts.tile([P, 1], f32)
        nc.vector.memset(halfpi[:], float(np.pi / 2.0))

        # ---------------- parameter pipeline (50-element, 1 partition) ------
        F1 = consts.tile([1, 3, N_PRIMES], f32)  # A-, B-, C-table on part. 0
        prm = consts.tile([1, N_PRIMES], i32)
        rs_sb = consts.tile([1, N_PRIMES], f32)
        ps_sb = consts.tile([1, N_PRIMES], f32)
        cnt_sb = consts.tile([1, N_PRIMES], f32)
        g8 = consts.tile([1, 8], f32)  # gAA gBB gCC 2gAB 2gAC 2gBC E -
        nc.vector.memset(g8[:], 0.0)

        nc.sync.dma_start(out=prm[:], in_=primes[None, :])
        nc.sync.dma_start(out=rs_sb[:], in_=rs[None, :])
        nc.sync.dma_start(out=ps_sb[:], in_=psh[None, :])
        nc.sync.dma_start(out=cnt_sb[:], in_=cnt_d[None, :])
        nc.sync.dma_start(out=g8[:, 6:7], in_=ep[None, :])

        pf = consts.tile([1, N_PRIMES], f32)
        nc.vector.tensor_copy(pf[:], prm[:])
        # sigmoid(x) = 0.5 + 0.5*tanh(x/2)   (keeps all ACT funcs in the
        # silu_and_others table set together with Sin -> one table load)
        sg = consts.tile([1, N_PRIMES], f32)
        nc.scalar.activation(sg[:], rs_sb[:], AF.Tanh, scale=0.5)
        nc.vector.tensor_scalar(sg[:], sg[:], 0.5, 0.5, op0=Op.mult, op1=Op.add)
        fA = F1[:, 0, :]
        fB = F1[:, 1, :]
        fC = F1[:, 2, :]
        nc.vector.tensor_tensor(fA, pf[:], sg[:], op=Op.mult)
        # fA *= BASE_SCALE * E
        nc.vector.tensor_scalar(fA, fA, BASE_SCALE, g8[:, 6:7], op0=Op.mult, op1=Op.mult)

        # sin/cos of phase shifts, range-reduced into [-pi, pi]:
        # x = ps/2pi; v = x - round(x) in [-1/2, 1/2] via the fp32 magic-add
        # trick; sin(ps) = Sin(2pi*v), cos(ps) = Sin(pi/2 - 2pi*|v|).
        sps = consts.tile([1, N_PRIMES], f32)
        cps = consts.tile([1, N_PRIMES], f32)
        tpp = consts.tile([1, N_PRIMES], f32)
        rpp = consts.tile([1, N_PRIMES], f32)
        upp = consts.tile([1, N_PRIMES], f32)
        aup = consts.tile([1, N_PRIMES], f32)
        nc.vector.tensor_scalar_mul(tpp[:], ps_sb[:], float(1.0 / (2 * np.pi)))
        nc.vector.tensor_scalar_add(rpp[:], tpp[:], MAGIC)
        nc.vector.tensor_scalar_add(rpp[:], rpp[:], -MAGIC)
        nc.vector.tensor_tensor(upp[:], tpp[:], rpp[:], op=Op.subtract)
        nc.vector.tensor_single_scalar(
            aup[:].bitcast(i32), upp[:].bitcast(i32), 0x7FFFFFFF, op=Op.bitwise_and
        )
        nc.scalar.activation(sps[:], upp[:], AF.Sin, scale=float(2 * np.pi))
        nc.scalar.activation(
            cps[:], aup[:], AF.Sin, scale=float(-2 * np.pi), bias=halfpi[0:1, :]
        )
        # fB = 0.1*fA*sin(ps), fC = 0.1*fA*cos(ps)
        nc.vector.scalar_tensor_tensor(fB, sps[:], 0.1, fA, op0=Op.mult, op1=Op.mult)
        nc.vector.scalar_tensor_tensor(fC, cps[:], 0.1, fA, op0=Op.mult, op1=Op.mult)

        # Gram scalars: g_XY = sum(cnt * fX * fY); cross terms carry the 2x.
        fAc = consts.tile([1, N_PRIMES], f32)
        fBc = consts.tile([1, N_PRIMES], f32)
        fCc = consts.tile([1, N_PRIMES], f32)
        nc.vector.tensor_tensor(fAc[:], fA, cnt_sb[:], op=Op.mult)
        nc.vector.tensor_tensor(fBc[:], fB, cnt_sb[:], op=Op.mult)
        nc.vector.tensor_tensor(fCc[:], fC, cnt_sb[:], op=Op.mult)
        gjunk = consts.tile([1, N_PRIMES], f32)
        for k, (x, yc, s) in enumerate(
            [
                (fA, fAc, 1.0),
                (fB, fBc, 1.0),
                (fC, fCc, 1.0),
                (fB, fAc, 2.0),
                (fC, fAc, 2.0),
                (fC, fBc, 2.0),
            ]
        ):
            nc.vector.scalar_tensor_tensor(
                gjunk[:], x, s, yc[:], op0=Op.mult, op1=Op.mult
            )
            nc.vector.tensor_reduce(
                g8[:, k : k + 1], gjunk[:], axis=mybir.AxisListType.X, op=Op.add
            )

        # broadcast g8 [1,8] -> [128,8] via a K=1 ones-matmul
        ones1 = consts.tile([1, P], f32)
        nc.vector.memset(ones1[:], 1.0)
        Gp = psum_t.tile([P, 8], f32, tag="Gp")
        nc.tensor.matmul(Gp[:], ones1[:], g8[:], start=True, stop=True)
        G = consts.tile([P, 8], f32)
        nc.vector.tensor_copy(G[:], Gp[:])

        # F3R: the [3,50] (A,B,C) table replicated at partitions 32g+k for
        # the four PE row groups; filled with 4 small SBUF->SBUF DMAs from F1.
        F3R = consts.tile([P, N_PRIMES], f32)
        F3Rv = F3R[:].rearrange("(g r) d -> g r d", r=32)
        for g in range(4):
            # src is the flat 150 elements on partition 0; the DMA respreads
            # them across 3 destination partitions
            nc.sync.dma_start(
                out=F3Rv[g, 0:3, :],
                in_=F1[:].rearrange("p k d -> p (k d)"),
            )

        # ---------------- per-row coefficients -----------------------------
        # token_hash laid out [128, 4]: element (p, t) = th[t*128 + p]
        thsb = rowp.tile([P, NT], i32)
        nc.sync.dma_start(out=thsb[:], in_=th[:].rearrange("(t p) -> p t", p=P))

        # exact m = th % 1000 via base-2^k folding (all products < 2^24,
        # exact in the DVE's fp32 ALU; shifts/ands are integer ops)
        mm = rowp.tile([P, NT], i32)
        sh = rowp.tile([P, NT], i32)
        lo = rowp.tile([P, NT], i32)
        cur = thsb
        for shift, mul, mask in [(16, 536, 65535)] + [(10, 24, 1023)] * 3:
            nc.vector.tensor_single_scalar(
                sh[:], cur[:], shift, op=Op.logical_shift_right
            )
            nc.vector.tensor_single_scalar(lo[:], cur[:], mask, op=Op.bitwise_and)
            nc.vector.tensor_single_scalar(sh[:], sh[:], mul, op=Op.mult)
            nc.vector.tensor_tensor(mm[:], sh[:], lo[:], op=Op.add)
            cur = mm
        ge = rowp.tile([P, NT], i32)
        nc.vector.tensor_single_scalar(ge[:], mm[:], 1000, op=Op.is_ge)
        nc.vector.scalar_tensor_tensor(
            mm[:], ge[:], -1000, mm[:], op0=Op.mult, op1=Op.add
        )
        mf = rowp.tile([P, NT], f32)
        nc.vector.tensor_copy(mf[:], mm[:])

        # sin/cos of 2*pi*tm, tm = m/1000 in [0,1); same magic-add reduction
        sinb = rowp.tile([P, NT], f32)
        cosb = rowp.tile([P, NT], f32)
        tpr = rowp.tile([P, NT], f32)
        rr = rowp.tile([P, NT], f32)
        ur = rowp.tile([P, NT], f32)
        aur = rowp.tile([P, NT], f32)
        nc.vector.tensor_scalar_mul(tpr[:], mf[:], 0.001)
        nc.vector.tensor_scalar_add(rr[:], tpr[:], MAGIC)
        nc.vector.tensor_scalar_add(rr[:], rr[:], -MAGIC)
        nc.vector.tensor_tensor(ur[:], tpr[:], rr[:], op=Op.subtract)
        nc.vector.tensor_single_scalar(
            aur[:].bitcast(i32), ur[:].bitcast(i32), 0x7FFFFFFF, op=Op.bitwise_and
        )
        nc.scalar.activation(sinb[:], ur[:], AF.Sin, scale=float(2 * np.pi))
        nc.scalar.activation(
            cosb[:], aur[:], AF.Sin, scale=float(-2 * np.pi), bias=halfpi[:]
        )

        # norm^2 = gAA + gBB c^2 + gCC s^2 + 2gAB c + 2gAC s + 2gBC s c
        w1 = rowp.tile([P, NT], f32)
        n2a = rowp.tile([P, NT], f32)
        w2 = rowp.tile([P, NT], f32)
        n2b = rowp.tile([P, NT], f32)
        sc = rowp.tile([P, NT], f32)
        n2c = rowp.tile([P, NT], f32)
        n2 = rowp.tile([P, NT], f32)
        nc.vector.tensor_scalar(
            w1[:], cosb[:], G[:, 1:2], G[:, 3:4], op0=Op.mult, op1=Op.add
        )
        nc.vector.tensor_tensor(n2a[:], w1[:], cosb[:], op=Op.mult)
        nc.vector.tensor_scalar(
            w2[:], sinb[:], G[:, 2:3], G[:, 4:5], op0=Op.mult, op1=Op.add
        )
        nc.vector.tensor_tensor(n2b[:], w2[:], sinb[:], op=Op.mult)
        nc.vector.tensor_tensor(sc[:], sinb[:], cosb[:], op=Op.mult)
        nc.vector.scalar_tensor_tensor(
            n2c[:], sc[:], G[:, 5:6], n2a[:], op0=Op.mult, op1=Op.add
        )
        nc.vector.tensor_tensor(n2[:], n2c[:], n2b[:], op=Op.add)
        nc.vector.tensor_scalar(
            n2[:], n2[:], G[:, 0:1], 1e-30, op0=Op.add, op1=Op.add
        )

        # rsqrt via bit-trick seed + 3 Newton iterations, all on the DVE
        rv = rowp.tile([P, NT], f32)
        rvt = rowp.tile([P, NT], f32)
        hf = rowp.tile([P, NT], f32)
        nc.vector.tensor_single_scalar(
            rvt[:].bitcast(i32), n2[:].bitcast(i32), 1, op=Op.logical_shift_right
        )
        nc.vector.tensor_scalar(
            rv[:].bitcast(i32),
            rvt[:].bitcast(i32),
            -1,
            0x5F3759DF,
            op0=Op.mult,
            op1=Op.add,
        )
        nc.vector.tensor_scalar_mul(hf[:], n2[:], 0.5)
        for _ in range(3):
            nc.vector.tensor_tensor(rvt[:], rv[:], rv[:], op=Op.mult)
            nc.vector.tensor_tensor(rvt[:], rvt[:], hf[:], op=Op.mult)
            nc.vector.tensor_scalar(rvt[:], rvt[:], -1.0, 1.5, op0=Op.mult, op1=Op.add)
            nc.vector.tensor_tensor(rv[:], rv[:], rvt[:], op=Op.mult)
        r = rowp.tile([P, NT], f32)
        nc.vector.tensor_scalar_mul(r[:], rv[:], G[:, 6:7])

        # pack [128, (t,k)] then transpose -> [12, 128]
        C12 = rowp.tile([P, NT, 3], f32)
        nc.vector.tensor_copy(C12[:, :, 0], r[:])
        nc.vector.tensor_tensor(C12[:, :, 1], cosb[:], r[:], op=Op.mult)
        nc.vector.tensor_tensor(C12[:, :, 2], sinb[:], r[:], op=Op.mult)

        Tp = psum_t.tile([NT * 3, P], f32)
        nc.tensor.transpose(Tp[:], C12[:].rearrange("p t k -> p (t k)"), ident[:])
        stage = rowp.tile([NT * 3, P], f32)
        nc.vector.tensor_copy(stage[:], Tp[:])

        # per-row-tile coefficients replicated at the four PE row groups
        coefRep = []
        for t in range(NT):
            cr = rowp.tile([P, P], f32, tag=f"coefRep{t}")
            crv = cr[:].rearrange("(g r) f -> g r f", r=32)
            for g in range(4):
                nc.sync.dma_start(out=crv[g, 0:3, :], in_=stage[3 * t : 3 * t + 3, :])
            coefRep.append(cr)

        # ---------------- main rank-3 matmul + store ------------------------
        # per row tile: 18 chunk matmuls packed 4-per-array via row groups;
        # rhs reads the [3,50] table with a stride-0 repeat AP.
        for t in range(NT):
            osb = outp.tile([P, DIM], f32)
            for ci, (off, ncols, reps) in enumerate(_CHUNKS):
                g = ci % 4
                pt = psum.tile([P, 512], f32)
                if reps is None:
                    rhs = F3Rv[g, 0:3, 0:ncols]
                else:
                    rhs = F3Rv[g, 0:3, None, :].broadcast_to([3, reps, N_PRIMES])
                nc.tensor.matmul(
                    pt[:, 0:ncols],
                    coefRep[t][32 * g : 32 * g + 3, :],
                    rhs,
                    start=True,
                    stop=True,
                    tile_position=(32 * g, 0),
                )
                dstv = osb[:, off : off + ncols]
                if ci % 2 == 0:
                    nc.scalar.copy(dstv, pt[:, 0:ncols])
                else:
                    nc.vector.tensor_copy(dstv, pt[:, 0:ncols])
            half = DIM // 2
            nc.sync.dma_start(
                out=out[t * P : (t + 1) * P, 0:half], in_=osb[:, 0:half]
            )
            nc.sync.dma_start(
                out=out[t * P : (t + 1) * P, half:DIM], in_=osb[:, half:DIM]
            )

    nc.compile()
    return nc


def _get_nc():
    if "nc" not in _CACHE:
        _CACHE["nc"] = _build()
    return _CACHE["nc"]


def kernel(
    token_hash,
    primes,
    resonance_scales,
    phase_shifts,
    energy_preservation,
    dimension,
):
    from concourse.bass_utils import run_bass_kernel_spmd

    token_hash = np.asarray(token_hash, dtype=np.int32)
    primes = np.asarray(primes, dtype=np.int32)
    resonance_scales = np.asarray(resonance_scales, dtype=np.float32)
    phase_shifts = np.asarray(phase_shifts, dtype=np.float32)
    ep = np.asarray(energy_preservation, dtype=np.float32).reshape(1)
    assert int(dimension) == DIM and token_hash.shape == (BATCH,)

    nc = _get_nc()
    in_maps = []
    for i in range(N_CORES):
        in_maps.append(
            {
                "token_hash": np.ascontiguousarray(
                    token_hash[i * B_LOC : (i + 1) * B_LOC]
                ),
                "primes": primes,
                "resonance_scales": resonance_scales,
                "phase_shifts": phase_shifts,
                "energy_preservation": ep,
            }
        )
    res = run_bass_kernel_spmd(nc, in_maps, core_ids=list(range(N_CORES)))
    return np.concatenate([r["out"] for r in res.results], axis=0)


# revision 29
# speedup vs baseline: 52.6196x; 52.6196x over previous
"""Trainium2 Bass kernel for PhysicalHarmonicResonanceSystem.

Math: spectrum[b,d] = freq[d] * (1 + 0.1*sin(ps[idx[d]] + 2*pi*tm_b)),
row-normalized to energy_preservation, idx = arange(D) % 50. Using the
angle-addition identity, each row is a rank-3 combination of three [D]
basis vectors:

    row_b = A + cos(th_b)*B + sin(th_b)*C
    A = freq, B = 0.1*freq*sin(ps[idx]), C = 0.1*freq*cos(ps[idx])

so ||row_b||^2 collapses to 6 Gram scalars of (A,B,C) (computed in
50-element space with occurrence counts), and the normalized output is a
[B,3] @ [3,D] matmul. The [3,D] basis is never materialized: since
idx repeats every 50 columns, the matmul's moving operand reads the
[3,50] table through a stride-0 access pattern in 500-column chunks.
The per-(row-tile, chunk) K=3 matmuls are packed 4-per-PE-array via
tile_position row groups. The kernel is bound by writing the 128MB
output to HBM (~47us/core at ~360GB/s).

Sharding: data-parallel over the batch dim, 512 rows per core x 8 cores.
"""

import numpy as np

N_PRIMES = 50
BATCH = 4096
DIM = 8192
N_CORES = 8
B_LOC = BATCH // N_CORES  # 512
P = 128
NT = B_LOC // P  # 4 row tiles per core
GOLDEN = (1.0 + np.sqrt(5.0)) / 2.0
BASE_SCALE = float(GOLDEN * np.pi / DIM)
MAGIC = float(1.5 * 2**23)  # fp32 round-to-nearest via add/sub

# matmul chunks: (col_offset, n_cols, n_pattern_reps or None for tail)
_CHUNKS = []
_off = 0
while _off + 500 <= DIM:
    _CHUNKS.append((_off, 500, 10))
    _off += 500
# tail: 8000..8192 = 192 cols = 3*50 + 42, pattern-aligned at 8000%50==0
_CHUNKS.append((_off, 150, 3))
_CHUNKS.append((_off + 150, 42, None))

_CACHE = {}


def _build(rep=1, loop_rep=None):
    from contextlib import ExitStack

    import concourse.bacc as bacc
    import concourse.bass as bass
    import concourse.tile as tile
    from concourse import mybir
    from concourse.masks import make_identity

    f32 = mybir.dt.float32
    i32 = mybir.dt.int32
    AF = mybir.ActivationFunctionType
    Op = mybir.AluOpType

    nc = bacc.Bacc()
    th = nc.dram_tensor("token_hash", [B_LOC], i32, kind="ExternalInput")
    primes = nc.dram_tensor("primes", [N_PRIMES], i32, kind="ExternalInput")
    rs = nc.dram_tensor("resonance_scales", [N_PRIMES], f32, kind="ExternalInput")
    psh = nc.dram_tensor("phase_shifts", [N_PRIMES], f32, kind="ExternalInput")
    ep = nc.dram_tensor("energy_preservation", [1], f32, kind="ExternalInput")
    out = nc.dram_tensor("out", [B_LOC, DIM], f32, kind="ExternalOutput")

    # occurrence count of each prime slot in idx = arange(DIM) % 50
    cnt = np.full(N_PRIMES, DIM // N_PRIMES, dtype=np.float32)
    cnt[: DIM % N_PRIMES] += 1.0
    cnt_d = nc.inline_tensor(cnt, name="cntw")

    with ExitStack() as ctx:
        tc = ctx.enter_context(tile.TileContext(nc))
        consts = ctx.enter_context(tc.tile_pool(name="consts", bufs=1))
        rowp = ctx.enter_context(tc.tile_pool(name="rowp", bufs=1))
        outp = ctx.enter_context(tc.tile_pool(name="outp", bufs=2))
        psum = ctx.enter_context(tc.tile_pool(name="psum", bufs=6, space="PSUM"))
        psum_t = ctx.enter_context(tc.tile_pool(name="psum_t", bufs=1, space="PSUM"))

        ident = consts.tile([P, P], f32)
        make_identity(nc, ident[:])
        halfpi = consts.tile([P, 1], f32)
        nc.vector.memset(halfpi[:], float(np.pi / 2.0))

        # ---------------- parameter pipeline (50-element, 1 partition) ------
        F1 = consts.tile([1, 3, N_PRIMES], f32)  # A-, B-, C-table on part. 0
        prm = consts.tile([1, N_PRIMES], i32)
        rs_sb = consts.tile([1, N_PRIMES], f32)
        ps_sb = consts.tile([1, N_PRIMES], f32)
        cnt_sb = consts.tile([1, N_PRIMES], f32)
        g8 = consts.tile([1, 8], f32)  # gAA gBB gCC 2gAB 2gAC 2gBC E -
        nc.vector.memset(g8[:], 0.0)

        nc.sync.dma_start(out=prm[:], in_=primes[None, :])
        nc.sync.dma_start(out=rs_sb[:], in_=rs[None, :])
        nc.sync.dma_start(out=ps_sb[:], in_=psh[None, :])
        nc.sync.dma_start(out=cnt_sb[:], in_=cnt_d[None, :])
        nc.sync.dma_start(out=g8[:, 6:7], in_=ep[None, :])

        pf = consts.tile([1, N_PRIMES], f32)
        nc.vector.tensor_copy(pf[:], prm[:])
        # sigmoid(x) = 0.5 + 0.5*tanh(x/2)   (keeps all ACT funcs in the
        # silu_and_others table set together with Sin -> one table load)
        sg = consts.tile([1, N_PRIMES], f32)
        nc.scalar.activation(sg[:], rs_sb[:], AF.Tanh, scale=0.5)
        nc.vector.tensor_scalar(sg[:], sg[:], 0.5, 0.5, op0=Op.mult, op1=Op.add)
        fA = F1[:, 0, :]
        fB = F1[:, 1, :]
        fC = F1[:, 2, :]
        nc.vector.tensor_tensor(fA, pf[:], sg[:], op=Op.mult)
        # fA *= BASE_SCALE * E
        nc.vector.tensor_scalar(fA, fA, BASE_SCALE, g8[:, 6:7], op0=Op.mult, op1=Op.mult)

        # sin/cos of phase shifts, range-reduced into [-pi, pi]:
        # x = ps/2pi; v = x - round(x) in [-1/2, 1/2] via the fp32 magic-add
        # trick; sin(ps) = Sin(2pi*v), cos(ps) = Sin(pi/2 - 2pi*|v|).
        sps = consts.tile([1, N_PRIMES], f32)
        cps = consts.tile([1, N_PRIMES], f32)
        tpp = consts.tile([1, N_PRIMES], f32)
        rpp = consts.tile([1, N_PRIMES], f32)
        upp = consts.tile([1, N_PRIMES], f32)
        aup = consts.tile([1, N_PRIMES], f32)
        nc.vector.tensor_scalar_mul(tpp[:], ps_sb[:], float(1.0 / (2 * np.pi)))
        nc.vector.tensor_scalar_add(rpp[:], tpp[:], MAGIC)
        nc.vector.tensor_scalar_add(rpp[:], rpp[:], -MAGIC)
        nc.vector.tensor_tensor(upp[:], tpp[:], rpp[:], op=Op.subtract)
        nc.vector.tensor_single_scalar(
            aup[:].bitcast(i32), upp[:].bitcast(i32), 0x7FFFFFFF, op=Op.bitwise_and
        )
        nc.scalar.activation(sps[:], upp[:], AF.Sin, scale=float(2 * np.pi))
        nc.scalar.activation(
            cps[:], aup[:], AF.Sin, scale=float(-2 * np.pi), bias=halfpi[0:1, :]
        )
        # fB = 0.1*fA*sin(ps), fC = 0.1*fA*cos(ps)
        nc.vector.scalar_tensor_tensor(fB, sps[:], 0.1, fA, op0=Op.mult, op1=Op.mult)
        nc.vector.scalar_tensor_tensor(fC, cps[:], 0.1, fA, op0=Op.mult, op1=Op.mult)

        # Gram scalars: g_XY = sum(cnt * fX * fY); cross terms carry the 2x.
        fAc = consts.tile([1, N_PRIMES], f32)
        fBc = consts.tile([1, N_PRIMES], f32)
        fCc = consts.tile([1, N_PRIMES], f32)
        nc.vector.tensor_tensor(fAc[:], fA, cnt_sb[:], op=Op.mult)
        nc.vector.tensor_tensor(fBc[:], fB, cnt_sb[:], op=Op.mult)
        nc.vector.tensor_tensor(fCc[:], fC, cnt_sb[:], op=Op.mult)
        gjunk = consts.tile([1, N_PRIMES], f32)
        for k, (x, yc, s) in enumerate(
            [
                (fA, fAc, 1.0),
                (fB, fBc, 1.0),
                (fC, fCc, 1.0),
                (fB, fAc, 2.0),
                (fC, fAc, 2.0),
                (fC, fBc, 2.0),
            ]
        ):
            nc.vector.scalar_tensor_tensor(
                gjunk[:], x, s, yc[:], op0=Op.mult, op1=Op.mult
            )
            nc.vector.tensor_reduce(
                g8[:, k : k + 1], gjunk[:], axis=mybir.AxisListType.X, op=Op.add
            )

        # broadcast g8 [1,8] -> [128,8] via a K=1 ones-matmul
        ones1 = consts.tile([1, P], f32)
        nc.vector.memset(ones1[:], 1.0)
        Gp = psum_t.tile([P, 8], f32, tag="Gp")
        nc.tensor.matmul(Gp[:], ones1[:], g8[:], start=True, stop=True)
        G = consts.tile([P, 8], f32)
        nc.vector.tensor_copy(G[:], Gp[:])

        # F3R: the [3,50] (A,B,C) table replicated at partitions 32g+k for
        # the four PE row groups; filled with 4 small SBUF->SBUF DMAs from F1.
        F3R = consts.tile([P, N_PRIMES], f32)
        F3Rv = F3R[:].rearrange("(g r) d -> g r d", r=32)
        for g in range(4):
            # src is the flat 150 elements on partition 0; the DMA respreads
            # them across 3 destination partitions
            nc.sync.dma_start(
                out=F3Rv[g, 0:3, :],
                in_=F1[:].rearrange("p k d -> p (k d)"),
            )

        # ---------------- per-row coefficients -----------------------------
        # token_hash laid out [128, 4]: element (p, t) = th[t*128 + p]
        thsb = rowp.tile([P, NT], i32)
        nc.sync.dma_start(out=thsb[:], in_=th[:].rearrange("(t p) -> p t", p=P))

        # exact m = th % 1000 via base-2^k folding (all products < 2^24,
        # exact in the DVE's fp32 ALU; shifts/ands are integer ops)
        mm = rowp.tile([P, NT], i32)
        sh = rowp.tile([P, NT], i32)
        lo = rowp.tile([P, NT], i32)
        cur = thsb
        for shift, mul, mask in [(16, 536, 65535)] + [(10, 24, 1023)] * 3:
            nc.vector.tensor_single_scalar(
                sh[:], cur[:], shift, op=Op.logical_shift_right
            )
            nc.vector.tensor_single_scalar(lo[:], cur[:], mask, op=Op.bitwise_and)
            nc.vector.tensor_single_scalar(sh[:], sh[:], mul, op=Op.mult)
            nc.vector.tensor_tensor(mm[:], sh[:], lo[:], op=Op.add)
            cur = mm
        ge = rowp.tile([P, NT], i32)
        nc.vector.tensor_single_scalar(ge[:], mm[:], 1000, op=Op.is_ge)
        nc.vector.scalar_tensor_tensor(
            mm[:], ge[:], -1000, mm[:], op0=Op.mult, op1=Op.add
        )
        mf = rowp.tile([P, NT], f32)
        nc.vector.tensor_copy(mf[:], mm[:])

        # sin/cos of 2*pi*tm, tm = m/1000 in [0,1); same magic-add reduction
        sinb = rowp.tile([P, NT], f32)
        cosb = rowp.tile([P, NT], f32)
        tpr = rowp.tile([P, NT], f32)
        rr = rowp.tile([P, NT], f32)
        ur = rowp.tile([P, NT], f32)
        aur = rowp.tile([P, NT], f32)
        nc.vector.tensor_scalar_mul(tpr[:], mf[:], 0.001)
        nc.vector.tensor_scalar_add(rr[:], tpr[:], MAGIC)
        nc.vector.tensor_scalar_add(rr[:], rr[:], -MAGIC)
        nc.vector.tensor_tensor(ur[:], tpr[:], rr[:], op=Op.subtract)
        nc.vector.tensor_single_scalar(
            aur[:].bitcast(i32), ur[:].bitcast(i32), 0x7FFFFFFF, op=Op.bitwise_and
        )
        nc.scalar.activation(sinb[:], ur[:], AF.Sin, scale=float(2 * np.pi))
        nc.scalar.activation(
            cosb[:], aur[:], AF.Sin, scale=float(-2 * np.pi), bias=halfpi[:]
        )

        # norm^2 = gAA + gBB c^2 + gCC s^2 + 2gAB c + 2gAC s + 2gBC s c
        w1 = rowp.tile([P, NT], f32)
        n2a = rowp.tile([P, NT], f32)
        w2 = rowp.tile([P, NT], f32)
        n2b = rowp.tile([P, NT], f32)
        sc = rowp.tile([P, NT], f32)
        n2c = rowp.tile([P, NT], f32)
        n2 = rowp.tile([P, NT], f32)
        nc.vector.tensor_scalar(
            w1[:], cosb[:], G[:, 1:2], G[:, 3:4], op0=Op.mult, op1=Op.add
        )
        nc.vector.tensor_tensor(n2a[:], w1[:], cosb[:], op=Op.mult)
        nc.vector.tensor_scalar(
            w2[:], sinb[:], G[:, 2:3], G[:, 4:5], op0=Op.mult, op1=Op.add
        )
        nc.vector.tensor_tensor(n2b[:], w2[:], sinb[:], op=Op.mult)
        nc.vector.tensor_tensor(sc[:], sinb[:], cosb[:], op=Op.mult)
        nc.vector.scalar_tensor_tensor(
            n2c[:], sc[:], G[:, 5:6], n2a[:], op0=Op.mult, op1=Op.add
        )
        nc.vector.tensor_tensor(n2[:], n2c[:], n2b[:], op=Op.add)
        nc.vector.tensor_scalar(
            n2[:], n2[:], G[:, 0:1], 1e-30, op0=Op.add, op1=Op.add
        )

        # rsqrt via bit-trick seed + 3 Newton iterations, all on the DVE
        rv = rowp.tile([P, NT], f32)
        rvt = rowp.tile([P, NT], f32)
        hf = rowp.tile([P, NT], f32)
        nc.vector.tensor_single_scalar(
            rvt[:].bitcast(i32), n2[:].bitcast(i32), 1, op=Op.logical_shift_right
        )
        nc.vector.tensor_scalar(
            rv[:].bitcast(i32),
            rvt[:].bitcast(i32),
            -1,
            0x5F3759DF,
            op0=Op.mult,
            op1=Op.add,
        )
        nc.vector.tensor_scalar_mul(hf[:], n2[:], 0.5)
        for _ in range(3):
            nc.vector.tensor_tensor(rvt[:], rv[:], rv[:], op=Op.mult)
            nc.vector.tensor_tensor(rvt[:], rvt[:], hf[:], op=Op.mult)
            nc.vector.tensor_scalar(rvt[:], rvt[:], -1.0, 1.5, op0=Op.mult, op1=Op.add)
            nc.vector.tensor_tensor(rv[:], rv[:], rvt[:], op=Op.mult)
        r = rowp.tile([P, NT], f32)
        nc.vector.tensor_scalar_mul(r[:], rv[:], G[:, 6:7])

        # pack [128, (t,k)] then transpose -> [12, 128]
        C12 = rowp.tile([P, NT, 3], f32)
        nc.vector.tensor_copy(C12[:, :, 0], r[:])
        nc.vector.tensor_tensor(C12[:, :, 1], cosb[:], r[:], op=Op.mult)
        nc.vector.tensor_tensor(C12[:, :, 2], sinb[:], r[:], op=Op.mult)

        Tp = psum_t.tile([NT * 3, P], f32)
        nc.tensor.transpose(Tp[:], C12[:].rearrange("p t k -> p (t k)"), ident[:])
        stage = rowp.tile([NT * 3, P], f32)
        nc.vector.tensor_copy(stage[:], Tp[:])

        # per-row-tile coefficients replicated at the four PE row groups
        coefRep = []
        for t in range(NT):
            cr = rowp.tile([P, P], f32, tag=f"coefRep{t}")
            crv = cr[:].rearrange("(g r) f -> g r f", r=32)
            for g in range(4):
                nc.sync.dma_start(out=crv[g, 0:3, :], in_=stage[3 * t : 3 * t + 3, :])
            coefRep.append(cr)

        # ---------------- main rank-3 matmul + store ------------------------
        # per row tile: 18 chunk matmuls packed 4-per-array via row groups;
        # rhs reads the [3,50] table with a stride-0 repeat AP.
        # rep>1 repeats the loop for steady-state benchmarking (same output).
        def main_section():
            for t in [tt % NT for tt in range(rep * NT)]:
                osb = outp.tile([P, DIM], f32)
                for ci, (off, ncols, reps) in enumerate(_CHUNKS):
                    g = ci % 4
                    pt = psum.tile([P, 512], f32)
                    if reps is None:
                        rhs = F3Rv[g, 0:3, 0:ncols]
                    else:
                        rhs = F3Rv[g, 0:3, None, :].broadcast_to([3, reps, N_PRIMES])
                    nc.tensor.matmul(
                        pt[:, 0:ncols],
                        coefRep[t][32 * g : 32 * g + 3, :],
                        rhs,
                        start=True,
                        stop=True,
                        tile_position=(32 * g, 0),
                    )
                    dstv = osb[:, off : off + ncols]
                    if ci % 2 == 0:
                        nc.scalar.copy(dstv, pt[:, 0:ncols])
                    else:
                        nc.vector.tensor_copy(dstv, pt[:, 0:ncols])
                half = DIM // 2
                nc.sync.dma_start(
                    out=out[t * P : (t + 1) * P, 0:half], in_=osb[:, 0:half]
                )
                nc.sync.dma_start(
                    out=out[t * P : (t + 1) * P, half:DIM], in_=osb[:, half:DIM]
                )

        if loop_rep is None:
            main_section()
        else:
            with tc.For_i(0, loop_rep, 1):
                main_section()

    nc.compile()
    return nc


def _get_nc(rep=1):
    key = ("nc", rep)
    if key not in _CACHE:
        _CACHE[key] = _build(rep)
    return _CACHE[key]


def kernel(
    token_hash,
    primes,
    resonance_scales,
    phase_shifts,
    energy_preservation,
    dimension,
):
    from concourse.bass_utils import run_bass_kernel_spmd

    token_hash = np.asarray(token_hash, dtype=np.int32)
    primes = np.asarray(primes, dtype=np.int32)
    resonance_scales = np.asarray(resonance_scales, dtype=np.float32)
    phase_shifts = np.asarray(phase_shifts, dtype=np.float32)
    ep = np.asarray(energy_preservation, dtype=np.float32).reshape(1)
    assert int(dimension) == DIM and token_hash.shape == (BATCH,)

    nc = _get_nc()
    in_maps = []
    for i in range(N_CORES):
        in_maps.append(
            {
                "token_hash": np.ascontiguousarray(
                    token_hash[i * B_LOC : (i + 1) * B_LOC]
                ),
                "primes": primes,
                "resonance_scales": resonance_scales,
                "phase_shifts": phase_shifts,
                "energy_preservation": ep,
            }
        )
    res = run_bass_kernel_spmd(nc, in_maps, core_ids=list(range(N_CORES)))
    return np.concatenate([r["out"] for r in res.results], axis=0)
